# revision 34
# baseline (speedup 1.0000x reference)
"""Multi-head causal attention + output projection on 8 Trainium2 cores.

Problem: B=4, T=2048, H=16, DQK=DV=64, E=1024, causal mask, fp32.

Sharding: core c -> batch b = c//2, head-group g = c%2 (8 heads each).
Each core computes full causal attention for its 8 heads and a partial
output projection (its heads' rows of W_o). Host sums the two partial
projections per batch and adds b_o.

Device algorithm (transposed layout, per head):
  scores^T(k,q) = K_h Q_h^T           (d on partitions; pre-transposed on host)
  attn^T = exp(scores^T * 1/8)        (ACT, no max-subtraction: scores ~ N(0,1))
  causal: structural tile skipping + triangular mask on diagonal tiles
  ctx'^T(65,q) = [V_h | 1]^T attn^T   (PSUM accumulate over k-tiles;
                                       row 64 = softmax denominators)
  ctx^T = ctx'^T[0:64] * (1/sums)     (one fused sel-matmul broadcast per
                                       head-pair + DVE mul from PSUM)
  out(q,E) = ctx^T.T @ W_o_rows       (lhsT=ctx^T, rhs=W_o natural)

All matmul operands are bf16 (full-rate on PE, half the DMA bytes of
f32r); PSUM accumulation stays fp32. The projection is interleaved one
matmul per attention k-step so PE never takes a long detour that
starves the ACT exp pipeline (ACT is the steady-state pacer).
"""

import numpy as np
import ml_dtypes

import concourse.bass as bass
import concourse.mybir as mybir
import concourse.tile as tile
from concourse import bacc
from concourse.bass_utils import run_bass_kernel_spmd

B, T, H, D, E = 4, 2048, 16, 64, 1024
HLOC = 8            # heads per core
NCORES = 8
TQ = 512            # q-block size
TK = 128            # k-tile size
NQB = T // TQ       # 4
NHP = HLOC // 2     # 4 head pairs
NKT = T // TK       # 16 k-tiles total
NQT = T // 128      # 16 output q-tiles
SCALE = 1.0 / np.sqrt(D)

F32 = mybir.dt.float32
F32R = mybir.dt.float32r
MM_DT = mybir.dt.bfloat16
BF16 = ml_dtypes.bfloat16

LABELS = {"PE": [], "ACT": [], "DVE": []}


def _lab(eng, s):
    LABELS[eng].append(s)


def _build_nc():
    nc = bacc.Bacc("TRN2", target_bir_lowering=False, debug=False,
                   num_devices=NCORES, name="mha")
    qt_d = nc.dram_tensor("qt", [HLOC * D, T], MM_DT, kind="ExternalInput")
    kt_d = nc.dram_tensor("kt", [HLOC * D, T], MM_DT, kind="ExternalInput")
    vo_d = nc.dram_tensor("vo", [NHP, T, 224], MM_DT, kind="ExternalInput")
    wo_d = nc.dram_tensor("wo", [HLOC * D, E], MM_DT, kind="ExternalInput")
    tri_d = nc.dram_tensor("tri", [TK, TK], MM_DT, kind="ExternalInput")
    sel_d = nc.dram_tensor("sel", [128, 128], F32R, kind="ExternalInput")
    out_d = nc.dram_tensor("out", [T, E], MM_DT, kind="ExternalOutput")

    EXP = mybir.ActivationFunctionType.Exp

    with tile.TileContext(nc) as tc:
        with (
            tc.tile_pool(name="const", bufs=1) as const_pool,
            tc.tile_pool(name="ctxT", bufs=1) as ctxT_pool,
            tc.tile_pool(name="qkt", bufs=1) as qkt_pool,
            tc.tile_pool(name="vsb", bufs=1) as v_pool,
            tc.tile_pool(name="attn", bufs=12) as attn_pool,
            tc.tile_pool(name="outsb", bufs=3) as out_pool,
            tc.tile_pool(name="bcs", bufs=2) as bcs_pool,
            tc.tile_pool(name="pp", bufs=2, space="PSUM") as pp_pool,
            tc.tile_pool(name="scores", bufs=2, space="PSUM") as scores_pool,
            tc.tile_pool(name="ctxA", bufs=1, space="PSUM") as ctxA_pool,
            tc.tile_pool(name="ctxB", bufs=1, space="PSUM") as ctxB_pool,
        ):
            tri_sb = const_pool.tile([TK, TK], MM_DT)
            sel_sb = const_pool.tile([128, 128], F32R)
            rc_sb = const_pool.tile([128, TQ], F32R)
            _lab("DVE", "memset")
            nc.vector.memset(rc_sb[:].bitcast(F32), 0.0)

            ctxT = ctxT_pool.tile([128, NHP, T], MM_DT)

            # all head-pairs resident in SBUF; load order puts hp=0 first so
            # attention starts as soon as the first chunks arrive
            kt_sbs, qt_sbs, v_sbs_all = [], [], []
            for hp in range(NHP):
                kt_sb = qkt_pool.tile([128, T], MM_DT, tag=f"kt{hp}", name="kt_sb")
                qt_sb = qkt_pool.tile([128, T], MM_DT, tag=f"qt{hp}", name="qt_sb")
                vP = v_pool.tile([128, NKT, 224], MM_DT, tag=f"vP{hp}", name="vP")
                kt_sbs.append(kt_sb)
                qt_sbs.append(qt_sb)
                v_sbs_all.append(vP)
            # chunked loads, first-needed first (q-blocks processed
            # ASCENDING): kt0's first k-tile, qt0's low block, tri (step 0
            # is a diagonal tile), first v chunk, then the rest
            for hp in range(NHP):
                kt_sb, qt_sb = kt_sbs[hp], qt_sbs[hp]
                vP = v_sbs_all[hp]
                hsl = slice(hp * 128, (hp + 1) * 128)
                qb0_ = 1          # first q-block processed
                qsl0 = slice(qb0_ * TQ, (qb0_ + 1) * TQ)
                if hp == 0:
                    nc.sync.dma_start(kt_sb[:, 0:TK], kt_d[hsl, 0:TK])
                    nc.sync.dma_start(qt_sb[:, qsl0], qt_d[hsl, qsl0])
                    nc.sync.dma_start(kt_sb[:, TK:512], kt_d[hsl, TK:512])
                    nc.sync.dma_start(tri_sb[:], tri_d[:])
                    nc.sync.dma_start(kt_sb[:, 512:1024], kt_d[hsl, 512:1024])
                else:
                    nc.sync.dma_start(kt_sb[:, 0:1024], kt_d[hsl, 0:1024])
                    nc.sync.dma_start(qt_sb[:, qsl0], qt_d[hsl, qsl0])
                nc.sync.dma_start(vP[:, 0:8], vo_d[hp].rearrange(
                    "(n p) m -> p n m", p=128)[:, 0:8])
                if hp == 0:
                    nc.sync.dma_start(sel_sb[:], sel_d[:])
                for ch in range(2, 4):
                    csl = slice(ch * 512, (ch + 1) * 512)
                    nc.sync.dma_start(kt_sb[:, csl], kt_d[hsl, csl])
                for ch in (2, 0, 3):   # remaining qt in block order
                    csl = slice(ch * TQ, (ch + 1) * TQ)
                    nc.sync.dma_start(qt_sb[:, csl], qt_d[hsl, csl])
                for ch in range(2, 4):
                    ksl = slice(ch * 4, (ch + 1) * 4)
                    nc.sync.dma_start(vP[:, ksl], vo_d[hp].rearrange(
                        "(n p) m -> p n m", p=128)[:, ksl])
            wo_sb = const_pool.tile([128, 4, E], MM_DT)
            nc.sync.dma_start(wo_sb[:], wo_d.rearrange("(n p) e -> p n e", p=128))

            # ---- interleaved output projection -------------------------
            # Each q-tile's projection is a generator yielding once per PE
            # matmul; the attention loop advances the queue one matmul per
            # k-step so projection fills PE slack without starving ACT.
            def gen_proj(qt_, tail=False):
                ot = out_pool.tile([128, E], MM_DT, tag="ot", name="ot")
                for eb in range(E // 512):
                    pp = pp_pool.tile([128, 512], F32, tag="pp", name="pp")
                    esl = slice(eb * 512, (eb + 1) * 512)
                    for kt_ in range(NHP):
                        _lab("PE", f"proj t{qt_}e{eb}k{kt_}")
                        nc.tensor.matmul(
                            pp[:],
                            lhsT=ctxT[:, kt_, qt_ * 128:(qt_ + 1) * 128],
                            rhs=wo_sb[:, kt_, esl],
                            start=(kt_ == 0), stop=(kt_ == NHP - 1),
                        )
                        if kt_ == NHP - 1:
                            # tail evacs alternate onto the (then idle) ACT
                            # engine so the drain is not DVE-serial
                            if tail and eb == 0:
                                nc.scalar.copy(ot[:, esl], pp[:])
                            else:
                                _lab("DVE", f"evac t{qt_}e{eb}")
                                nc.vector.tensor_copy(ot[:, esl], pp[:])
                            nc.sync.dma_start(
                                out_d[qt_ * 128:(qt_ + 1) * 128, esl],
                                ot[:, esl])
                        yield

            proj_q = []

            def pop_proj(n):
                while n > 0 and proj_q:
                    try:
                        next(proj_q[0])
                        n -= 1
                    except StopIteration:
                        proj_q.pop(0)

            def emit_qk_exp(qb, hp, kk, nfull):
                q0 = max(kk - nfull, 0) * TK
                kt_sb, qt_sb = kt_sbs[hp], qt_sbs[hp]
                scr = scores_pool.tile([128, 2, TQ], F32, tag="scr", name="scr")
                at = attn_pool.tile([128, 2, TQ], MM_DT, tag="attn", name="attn")
                # adjacent QK matmuls on disjoint row groups overlap
                for head in (0, 1):
                    dr = slice(head * D, head * D + D)
                    _lab("PE", f"QK q{qb}p{hp}k{kk}h{head}")
                    nc.tensor.matmul(
                        scr[:, head, q0:TQ],
                        lhsT=kt_sb[dr, kk * TK:(kk + 1) * TK],
                        rhs=qt_sb[dr, qb * TQ + q0:(qb + 1) * TQ],
                        start=True, stop=True,
                    )
                _lab("ACT", f"exp q{qb}p{hp}k{kk}")
                nc.scalar.activation(at[:, :, q0:TQ], scr[:, :, q0:TQ],
                                     EXP, scale=float(SCALE))
                if kk >= nfull:
                    # causal mask on the diagonal tile: run on the otherwise
                    # idle gpsimd engine (SBUF-only, standard tensor op) to
                    # keep DVE's queue short for the norm chains
                    for head in (0, 1):
                        nc.gpsimd.tensor_mul(at[:, head, q0:q0 + TK],
                                             at[:, head, q0:q0 + TK], tri_sb[:])
                return at, q0

            def emit_pv(hp, kk, nk, at, q0, ctx_ts):
                for head in (0, 1):
                    # PV + sums in one M=128 matmul:
                    # even head: [V|1|0..] -> ctx 0:64, sums row 64
                    # odd head:  [0..|1@32|V] -> sums row 32, ctx 64:128
                    _lab("PE", f"PV k{kk}h{head}")
                    nc.tensor.matmul(
                        ctx_ts[head][:, q0:TQ],
                        lhsT=(v_sbs_all[hp][:, kk, 0:128] if head == 0
                              else v_sbs_all[hp][:, kk, 96:224]),
                        rhs=at[:, head, q0:TQ],
                        start=(kk == 0), stop=(kk == nk - 1),
                    )

            # ---- flat software-pipelined schedule ----------------------
            # One global stream of k-steps across all (qb, hp): QK+exp for
            # step s is emitted DELTA steps before its PV, so ACT always has
            # a DELTA-deep runway of pending exps and never starves at pair
            # boundaries. Norm pieces are scheduled as inter-step closures a
            # couple of steps after each pair's final PV; with DELTA=6 the
            # next pair's first PV lands well after the norm multiplies have
            # freed the ctx PSUM banks, so PE never blocks on them.
            DELTA = 8

            # Pair order: qb1 first (shortest no-projection prefix that is
            # not norm-latency-bound); qb0's short norm-congested pairs are
            # sandwiched between qb2's long ones so the deep pipeline of the
            # long pairs absorbs qb0's norm latency; qb3 last (its
            # projections form the unavoidable tail).
            QB_ORDER = (1, 2, 0, 3)
            pair_list = [(1, hp) for hp in range(NHP)]
            for hp in range(NHP):
                pair_list.append((2, hp))
                pair_list.append((0, hp))
            pair_list.extend((3, hp) for hp in range(NHP))
            steps = []
            for qb, hp in pair_list:
                nk = (qb + 1) * (TQ // TK)
                nfull = nk - (TQ // TK)
                for kk in range(nk):
                    steps.append((qb, hp, kk, nk, nfull))
            NSTEP = len(steps)

            def emit_norm_recips(ctx_ts):
                _lab("DVE", "recip0"); _lab("DVE", "recip1")
                with nc.allow_low_precision(reason="f32r recips"):
                    nc.vector.reciprocal(rc_sb[D:D + 1], ctx_ts[0][D:D + 1])
                    nc.vector.reciprocal(rc_sb[32:33], ctx_ts[1][32:33])

            def emit_norm_bcast():
                bc = pp_pool.tile([128, TQ], F32, tag="pp", name="bc")
                _lab("PE", "selmm")
                nc.tensor.matmul(bc[:], lhsT=sel_sb[:], rhs=rc_sb[:],
                                 start=True, stop=True)
                # DVE cannot read two PSUM operands: stage bc in SBUF
                bcs = bcs_pool.tile([128, TQ], F32, tag="bcs", name="bcs")
                _lab("DVE", "bccopy")
                nc.vector.tensor_copy(bcs[:], bc[:])
                return bcs

            def emit_norm_muls(hp, qsl, ctx_ts, bcs):
                for head in (0, 1):
                    hsl2 = slice(0, D) if head == 0 else slice(D, 128)
                    _lab("DVE", f"normmul h{head}")
                    nc.vector.tensor_mul(ctxT[hsl2, hp, qsl],
                                         ctx_ts[head][hsl2], bcs[hsl2])

            post = {}           # step index -> [closures]

            def at_step(s, f):
                post.setdefault(s, []).append(f)

            cur_ctx = [None]    # ctx PSUM tiles of the pair currently in PV
            pend = {}           # step -> (at, q0) awaiting PV

            for s in range(NSTEP + DELTA):
                if s < NSTEP:
                    qb, hp, kk, nk, nfull = steps[s]
                    pend[s] = emit_qk_exp(qb, hp, kk, nfull)
                for f in post.pop(s, ()):
                    f()
                pop_proj(2 if (s < NSTEP and steps[s][2] >= steps[s][4])
                         else 1)
                sp = s - DELTA
                if sp < 0:
                    continue
                qb2, hp2, kk2, nk2, nfull2 = steps[sp]
                at2, q02 = pend.pop(sp)
                if kk2 == 0:
                    cur_ctx[0] = (ctxA_pool.tile([128, TQ], F32, tag="ctxA",
                                                 name="ctxA"),
                                  ctxB_pool.tile([128, TQ], F32, tag="ctxB",
                                                 name="ctxB"))
                emit_pv(hp2, kk2, nk2, at2, q02, cur_ctx[0])
                if kk2 == nk2 - 1:
                    ctx_ts = cur_ctx[0]
                    qsl2 = slice(qb2 * TQ, (qb2 + 1) * TQ)
                    emit_norm_recips(ctx_ts)

                    def mk(hp2=hp2, qb2=qb2, qsl2=qsl2, ctx_ts=ctx_ts):
                        def _bcast_muls():
                            bcs = emit_norm_bcast()
                            emit_norm_muls(hp2, qsl2, ctx_ts, bcs)
                            if hp2 == NHP - 1:
                                proj_q.extend(
                                    gen_proj(qt_, tail=(qb2 == QB_ORDER[-1]))
                                    for qt_ in range(
                                        qb2 * (TQ // 128),
                                        (qb2 + 1) * (TQ // 128)))
                        at_step(s + 1, _bcast_muls)
                    mk()

            while post:
                ss = min(post)
                for f2 in post.pop(ss):
                    f2()
            while proj_q:
                pop_proj(1 << 30)

    nc.compile()
    return nc


_NC_CACHE = {}


def _get_nc():
    if "nc" not in _NC_CACHE:
        _NC_CACHE["nc"] = _build_nc()
    return _NC_CACHE["nc"]


def build_in_maps(Q, K, V, W_o):
    # transposed layout [k partitions, q free]: valid iff k <= q
    tri = np.triu(np.ones((TK, TK), dtype=np.float32)).astype(BF16)
    sel = np.zeros((128, 128), dtype=np.float32)
    sel[D, 0:D] = 1.0     # head even: broadcast recip row 64 to rows 0:64
    sel[32, D:128] = 1.0  # head odd: broadcast recip row 32 to rows 64:128

    in_maps = []
    for c in range(NCORES):
        b, g = c // 2, c % 2
        hs = slice(g * HLOC * D, (g + 1) * HLOC * D)
        qt = np.ascontiguousarray(Q[b][:, hs].T).astype(BF16)   # (512, 2048)
        kt = np.ascontiguousarray(K[b][:, hs].T).astype(BF16)
        # packed pair stationary, 224 cols: even head reads cols [0:128]
        # = [V_e|1@64|0..], odd head reads [96:224] = [0..|1@32|0..|V_o]
        vo = np.zeros((NHP, T, 224), dtype=np.float32)
        for hp in range(NHP):
            ve = V[b][:, (g * HLOC + 2 * hp) * D:(g * HLOC + 2 * hp + 1) * D]
            vo_ = V[b][:, (g * HLOC + 2 * hp + 1) * D:(g * HLOC + 2 * hp + 2) * D]
            vo[hp, :, 0:D] = ve
            vo[hp, :, D] = 1.0        # even head sums col -> psum row 64
            vo[hp, :, 128] = 1.0      # odd head local col 32 -> psum row 32
            vo[hp, :, 160:224] = vo_
        wo = np.ascontiguousarray(W_o[hs, :]).astype(BF16)      # (512, 1024)
        in_maps.append({"qt": qt, "kt": kt, "vo": vo.astype(BF16), "wo": wo,
                        "tri": tri, "sel": sel})
    return in_maps


def _kernel_numpy(Q, K, V, mask, W_o, b_o):
    """Reference fallback for non-causal masks (never hit in practice)."""
    out = np.empty((B, T, E), dtype=np.float32)
    for b in range(B):
        q = Q[b].reshape(T, H, D).transpose(1, 0, 2)
        k = K[b].reshape(T, H, D).transpose(1, 0, 2)
        v = V[b].reshape(T, H, D).transpose(1, 0, 2)
        s = np.einsum("hqd,hkd->hqk", q, k) / np.sqrt(D)
        s = np.where(mask[b][None], -np.inf, s)
        a = np.exp(s - s.max(-1, keepdims=True))
        a /= a.sum(-1, keepdims=True)
        ctx = np.einsum("hqk,hkd->hqd", a, v).transpose(1, 0, 2).reshape(T, H * D)
        out[b] = ctx @ W_o + b_o
    return out


_CAUSAL = None


def _is_causal(mask):
    global _CAUSAL
    if _CAUSAL is None:
        _CAUSAL = np.triu(np.ones((T, T), dtype=bool), 1)
    m = np.asarray(mask)
    return m.shape == (B, T, T) and all(np.array_equal(m[b], _CAUSAL) for b in range(B))


def kernel(Q, K, V, mask, W_o, b_o):
    Q = np.asarray(Q, dtype=np.float32)
    K = np.asarray(K, dtype=np.float32)
    V = np.asarray(V, dtype=np.float32)
    W_o = np.asarray(W_o, dtype=np.float32)
    b_o = np.asarray(b_o, dtype=np.float32)

    if not _is_causal(mask):
        return _kernel_numpy(Q, K, V, np.asarray(mask, dtype=bool), W_o, b_o)

    in_maps = build_in_maps(Q, K, V, W_o)

    nc = _get_nc()
    res = run_bass_kernel_spmd(nc, in_maps, core_ids=list(range(NCORES)))
    _NC_CACHE["last_results"] = res

    out = np.empty((B, T, E), dtype=np.float32)
    for b in range(B):
        out[b] = (res.results[2 * b]["out"].astype(np.float32)
                  + res.results[2 * b + 1]["out"].astype(np.float32))
    out += b_o
    return out


# revision 36
# speedup vs baseline: 1.0331x; 1.0331x over previous
"""Multi-head causal attention + output projection on 8 Trainium2 cores.

Problem: B=4, T=2048, H=16, DQK=DV=64, E=1024, causal mask, fp32.

Sharding: core c -> batch b = c//2, head-group g = c%2 (8 heads each).
Each core computes full causal attention for its 8 heads and a partial
output projection (its heads' rows of W_o). Host sums the two partial
projections per batch and adds b_o.

Device algorithm (transposed layout, per head):
  scores^T(k,q) = K_h Q_h^T           (d on partitions; pre-transposed on host)
  attn^T = exp(scores^T * 1/8)        (ACT, no max-subtraction: scores ~ N(0,1))
  causal: structural tile skipping + triangular mask on diagonal tiles
  ctx'^T(65,q) = [V_h | 1]^T attn^T   (PSUM accumulate over k-tiles;
                                       row 64 = softmax denominators)
  ctx^T = ctx'^T[0:64] * (1/sums)     (one fused sel-matmul broadcast per
                                       head-pair + DVE mul from PSUM)
  out(q,E) = ctx^T.T @ W_o_rows       (lhsT=ctx^T, rhs=W_o natural)

All matmul operands are bf16 (full-rate on PE, half the DMA bytes of
f32r); PSUM accumulation stays fp32. The projection is interleaved one
matmul per attention k-step so PE never takes a long detour that
starves the ACT exp pipeline (ACT is the steady-state pacer).
"""

import numpy as np
import ml_dtypes

import concourse.bass as bass
import concourse.mybir as mybir
import concourse.tile as tile
from concourse import bacc
from concourse.bass_utils import run_bass_kernel_spmd

B, T, H, D, E = 4, 2048, 16, 64, 1024
HLOC = 8            # heads per core
NCORES = 8
TQ = 512            # q-block size
TK = 128            # k-tile size
NQB = T // TQ       # 4
NHP = HLOC // 2     # 4 head pairs
NKT = T // TK       # 16 k-tiles total
NQT = T // 128      # 16 output q-tiles
SCALE = 1.0 / np.sqrt(D)

F32 = mybir.dt.float32
F32R = mybir.dt.float32r
MM_DT = mybir.dt.bfloat16
BF16 = ml_dtypes.bfloat16

LABELS = {"PE": [], "ACT": [], "DVE": []}


def _lab(eng, s):
    LABELS[eng].append(s)


def _build_nc():
    nc = bacc.Bacc("TRN2", target_bir_lowering=False, debug=False,
                   num_devices=NCORES, name="mha")
    qt_d = nc.dram_tensor("qt", [HLOC * D, T], MM_DT, kind="ExternalInput")
    kt_d = nc.dram_tensor("kt", [HLOC * D, T], MM_DT, kind="ExternalInput")
    vo_d = nc.dram_tensor("vo", [NHP, T, 224], MM_DT, kind="ExternalInput")
    wo_d = nc.dram_tensor("wo", [HLOC * D, E], MM_DT, kind="ExternalInput")
    tri_d = nc.dram_tensor("tri", [TK, TK], MM_DT, kind="ExternalInput")
    sel_d = nc.dram_tensor("sel", [128, 128], F32R, kind="ExternalInput")
    out_d = nc.dram_tensor("out", [T, E], MM_DT, kind="ExternalOutput")

    EXP = mybir.ActivationFunctionType.Exp

    with tile.TileContext(nc) as tc:
        with (
            tc.tile_pool(name="const", bufs=1) as const_pool,
            tc.tile_pool(name="ctxT", bufs=1) as ctxT_pool,
            tc.tile_pool(name="qkt", bufs=1) as qkt_pool,
            tc.tile_pool(name="vsb", bufs=1) as v_pool,
            tc.tile_pool(name="attn", bufs=12) as attn_pool,
            tc.tile_pool(name="outsb", bufs=3) as out_pool,
            tc.tile_pool(name="bcs", bufs=2) as bcs_pool,
            tc.tile_pool(name="pp", bufs=2, space="PSUM") as pp_pool,
            tc.tile_pool(name="scores", bufs=2, space="PSUM") as scores_pool,
            tc.tile_pool(name="ctxA", bufs=1, space="PSUM") as ctxA_pool,
            tc.tile_pool(name="ctxB", bufs=1, space="PSUM") as ctxB_pool,
        ):
            tri_sb = const_pool.tile([TK, TK], MM_DT)
            sel_sb = const_pool.tile([128, 128], F32R)
            rc_sb = const_pool.tile([128, TQ], F32R)
            _lab("DVE", "memset")
            nc.vector.memset(rc_sb[:].bitcast(F32), 0.0)

            ctxT = ctxT_pool.tile([128, NHP, T], MM_DT)

            # all head-pairs resident in SBUF; load order puts hp=0 first so
            # attention starts as soon as the first chunks arrive
            kt_sbs, qt_sbs, v_sbs_all = [], [], []
            for hp in range(NHP):
                kt_sb = qkt_pool.tile([128, T], MM_DT, tag=f"kt{hp}", name="kt_sb")
                qt_sb = qkt_pool.tile([128, T], MM_DT, tag=f"qt{hp}", name="qt_sb")
                vP = v_pool.tile([128, NKT, 224], MM_DT, tag=f"vP{hp}", name="vP")
                kt_sbs.append(kt_sb)
                qt_sbs.append(qt_sb)
                v_sbs_all.append(vP)
            # chunked loads, first-needed first (q-blocks processed
            # ASCENDING): kt0's first k-tile, qt0's low block, tri (step 0
            # is a diagonal tile), first v chunk, then the rest
            for hp in range(NHP):
                kt_sb, qt_sb = kt_sbs[hp], qt_sbs[hp]
                vP = v_sbs_all[hp]
                hsl = slice(hp * 128, (hp + 1) * 128)
                qb0_ = 1          # first q-block processed
                qsl0 = slice(qb0_ * TQ, (qb0_ + 1) * TQ)
                if hp == 0:
                    nc.sync.dma_start(kt_sb[:, 0:TK], kt_d[hsl, 0:TK])
                    nc.sync.dma_start(qt_sb[:, qsl0], qt_d[hsl, qsl0])
                    nc.sync.dma_start(kt_sb[:, TK:512], kt_d[hsl, TK:512])
                    nc.sync.dma_start(tri_sb[:], tri_d[:])
                    nc.sync.dma_start(kt_sb[:, 512:1024], kt_d[hsl, 512:1024])
                else:
                    nc.sync.dma_start(kt_sb[:, 0:1024], kt_d[hsl, 0:1024])
                    nc.sync.dma_start(qt_sb[:, qsl0], qt_d[hsl, qsl0])
                nc.sync.dma_start(vP[:, 0:8], vo_d[hp].rearrange(
                    "(n p) m -> p n m", p=128)[:, 0:8])
                if hp == 0:
                    nc.sync.dma_start(sel_sb[:], sel_d[:])
                for ch in range(2, 4):
                    csl = slice(ch * 512, (ch + 1) * 512)
                    nc.sync.dma_start(kt_sb[:, csl], kt_d[hsl, csl])
                for ch in (2, 0, 3):   # remaining qt in block order
                    csl = slice(ch * TQ, (ch + 1) * TQ)
                    nc.sync.dma_start(qt_sb[:, csl], qt_d[hsl, csl])
                for ch in range(2, 4):
                    ksl = slice(ch * 4, (ch + 1) * 4)
                    nc.sync.dma_start(vP[:, ksl], vo_d[hp].rearrange(
                        "(n p) m -> p n m", p=128)[:, ksl])
            wo_sb = const_pool.tile([128, 4, E], MM_DT)
            nc.sync.dma_start(wo_sb[:], wo_d.rearrange("(n p) e -> p n e", p=128))

            # ---- interleaved output projection -------------------------
            # Each q-tile's projection is a generator yielding once per PE
            # matmul; the attention loop advances the queue one matmul per
            # k-step so projection fills PE slack without starving ACT.
            def gen_proj(qt_, tail=False):
                ot = out_pool.tile([128, E], MM_DT, tag="ot", name="ot")
                for eb in range(E // 512):
                    pp = pp_pool.tile([128, 512], F32, tag="pp", name="pp")
                    esl = slice(eb * 512, (eb + 1) * 512)
                    for kt_ in range(NHP):
                        _lab("PE", f"proj t{qt_}e{eb}k{kt_}")
                        nc.tensor.matmul(
                            pp[:],
                            lhsT=ctxT[:, kt_, qt_ * 128:(qt_ + 1) * 128],
                            rhs=wo_sb[:, kt_, esl],
                            start=(kt_ == 0), stop=(kt_ == NHP - 1),
                        )
                        if kt_ == NHP - 1:
                            # tail evacs alternate onto the (then idle) ACT
                            # engine so the drain is not DVE-serial
                            if tail and eb == 0:
                                nc.scalar.copy(ot[:, esl], pp[:])
                            else:
                                _lab("DVE", f"evac t{qt_}e{eb}")
                                nc.vector.tensor_copy(ot[:, esl], pp[:])
                            nc.sync.dma_start(
                                out_d[qt_ * 128:(qt_ + 1) * 128, esl],
                                ot[:, esl])
                        yield

            proj_q = []

            def pop_proj(n):
                while n > 0 and proj_q:
                    try:
                        next(proj_q[0])
                        n -= 1
                    except StopIteration:
                        proj_q.pop(0)

            def emit_qk_exp(qb, hp, kk, nfull):
                q0 = max(kk - nfull, 0) * TK
                kt_sb, qt_sb = kt_sbs[hp], qt_sbs[hp]
                scr = scores_pool.tile([128, 2, TQ], F32, tag="scr", name="scr")
                at = attn_pool.tile([128, 2, TQ], MM_DT, tag="attn", name="attn")
                # adjacent QK matmuls on disjoint row groups overlap
                for head in (0, 1):
                    dr = slice(head * D, head * D + D)
                    _lab("PE", f"QK q{qb}p{hp}k{kk}h{head}")
                    nc.tensor.matmul(
                        scr[:, head, q0:TQ],
                        lhsT=kt_sb[dr, kk * TK:(kk + 1) * TK],
                        rhs=qt_sb[dr, qb * TQ + q0:(qb + 1) * TQ],
                        start=True, stop=True,
                    )
                _lab("ACT", f"exp q{qb}p{hp}k{kk}")
                nc.scalar.activation(at[:, :, q0:TQ], scr[:, :, q0:TQ],
                                     EXP, scale=float(SCALE))
                if kk >= nfull:
                    for head in (0, 1):
                        _lab("DVE", f"tri q{qb}p{hp}k{kk}h{head}")
                        nc.vector.tensor_mul(at[:, head, q0:q0 + TK],
                                             at[:, head, q0:q0 + TK], tri_sb[:])
                return at, q0

            def emit_pv(hp, kk, nk, at, q0, ctx_ts):
                for head in (0, 1):
                    # PV + sums in one M=128 matmul:
                    # even head: [V|1|0..] -> ctx 0:64, sums row 64
                    # odd head:  [0..|1@32|V] -> sums row 32, ctx 64:128
                    _lab("PE", f"PV k{kk}h{head}")
                    nc.tensor.matmul(
                        ctx_ts[head][:, q0:TQ],
                        lhsT=(v_sbs_all[hp][:, kk, 0:128] if head == 0
                              else v_sbs_all[hp][:, kk, 96:224]),
                        rhs=at[:, head, q0:TQ],
                        start=(kk == 0), stop=(kk == nk - 1),
                    )

            # ---- flat software-pipelined schedule ----------------------
            # One global stream of k-steps across all (qb, hp): QK+exp for
            # step s is emitted DELTA steps before its PV, so ACT always has
            # a DELTA-deep runway of pending exps and never starves at pair
            # boundaries. Norm pieces are scheduled as inter-step closures a
            # couple of steps after each pair's final PV; with DELTA=6 the
            # next pair's first PV lands well after the norm multiplies have
            # freed the ctx PSUM banks, so PE never blocks on them.
            DELTA = 8

            # Pair order: qb1 first (shortest no-projection prefix that is
            # not norm-latency-bound); qb0's short norm-congested pairs are
            # sandwiched between qb2's long ones so the deep pipeline of the
            # long pairs absorbs qb0's norm latency; qb3 last (its
            # projections form the unavoidable tail).
            QB_ORDER = (1, 2, 0, 3)
            pair_list = [(1, hp) for hp in range(NHP)]
            for hp in range(NHP):
                pair_list.append((2, hp))
                pair_list.append((0, hp))
            pair_list.extend((3, hp) for hp in range(NHP))
            steps = []
            for qb, hp in pair_list:
                nk = (qb + 1) * (TQ // TK)
                nfull = nk - (TQ // TK)
                for kk in range(nk):
                    steps.append((qb, hp, kk, nk, nfull))
            NSTEP = len(steps)

            def emit_norm_recips(ctx_ts):
                _lab("DVE", "recip0"); _lab("DVE", "recip1")
                with nc.allow_low_precision(reason="f32r recips"):
                    nc.vector.reciprocal(rc_sb[D:D + 1], ctx_ts[0][D:D + 1])
                    nc.vector.reciprocal(rc_sb[32:33], ctx_ts[1][32:33])

            def emit_norm_bcast():
                bc = pp_pool.tile([128, TQ], F32, tag="pp", name="bc")
                _lab("PE", "selmm")
                nc.tensor.matmul(bc[:], lhsT=sel_sb[:], rhs=rc_sb[:],
                                 start=True, stop=True)
                # DVE cannot read two PSUM operands: stage bc in SBUF
                bcs = bcs_pool.tile([128, TQ], F32, tag="bcs", name="bcs")
                _lab("DVE", "bccopy")
                nc.vector.tensor_copy(bcs[:], bc[:])
                return bcs

            def emit_norm_muls(hp, qsl, ctx_ts, bcs):
                for head in (0, 1):
                    hsl2 = slice(0, D) if head == 0 else slice(D, 128)
                    _lab("DVE", f"normmul h{head}")
                    nc.vector.tensor_mul(ctxT[hsl2, hp, qsl],
                                         ctx_ts[head][hsl2], bcs[hsl2])

            post = {}           # step index -> [closures]

            def at_step(s, f):
                post.setdefault(s, []).append(f)

            cur_ctx = [None]    # ctx PSUM tiles of the pair currently in PV
            pend = {}           # step -> (at, q0) awaiting PV

            # PE budget model: keep PE's queued work per step just under
            # ACT's exp cadence (~1038ns/step) so the in-order PE queue
            # never delays the next QK past the moment ACT needs it. Each
            # emitted matmul adds its model time; each step grants 1000ns.
            ACT_STEP = 1000.0
            debt = [0.0]

            def pe_cost(ncols):
                return ncols * 0.4167

            for s in range(NSTEP + DELTA):
                if s < NSTEP:
                    qb, hp, kk, nk, nfull = steps[s]
                    pend[s] = emit_qk_exp(qb, hp, kk, nfull)
                    debt[0] += pe_cost(2 * (TQ - max(kk - nfull, 0) * TK))
                for f in post.pop(s, ()):
                    f()
                budget = (ACT_STEP if s < NSTEP else 10000.0) - debt[0]
                while budget > 0 and proj_q:
                    pop_proj(1)
                    debt[0] += pe_cost(512)
                    budget -= pe_cost(512)
                sp = s - DELTA
                debt[0] = max(debt[0] - (ACT_STEP if s < NSTEP else 10000.0),
                              0.0)
                if sp < 0:
                    continue
                qb2, hp2, kk2, nk2, nfull2 = steps[sp]
                at2, q02 = pend.pop(sp)
                if kk2 == 0:
                    cur_ctx[0] = (ctxA_pool.tile([128, TQ], F32, tag="ctxA",
                                                 name="ctxA"),
                                  ctxB_pool.tile([128, TQ], F32, tag="ctxB",
                                                 name="ctxB"))
                emit_pv(hp2, kk2, nk2, at2, q02, cur_ctx[0])
                debt[0] += pe_cost(2 * (TQ - q02))
                if kk2 == nk2 - 1:
                    ctx_ts = cur_ctx[0]
                    qsl2 = slice(qb2 * TQ, (qb2 + 1) * TQ)
                    emit_norm_recips(ctx_ts)

                    def mk(hp2=hp2, qb2=qb2, qsl2=qsl2, ctx_ts=ctx_ts):
                        def _bcast_muls():
                            # at s+3 the recips are long done, so the sel
                            # matmul never stalls the in-order PE queue
                            bcs = emit_norm_bcast()
                            debt[0] += pe_cost(512)
                            emit_norm_muls(hp2, qsl2, ctx_ts, bcs)
                            if hp2 == NHP - 1:
                                proj_q.extend(
                                    gen_proj(qt_, tail=(qb2 == QB_ORDER[-1]))
                                    for qt_ in range(
                                        qb2 * (TQ // 128),
                                        (qb2 + 1) * (TQ // 128)))
                        at_step(s + 3, _bcast_muls)
                    mk()

            while post:
                ss = min(post)
                for f2 in post.pop(ss):
                    f2()
            while proj_q:
                pop_proj(1 << 30)

    nc.compile()
    return nc


_NC_CACHE = {}


def _get_nc():
    if "nc" not in _NC_CACHE:
        _NC_CACHE["nc"] = _build_nc()
    return _NC_CACHE["nc"]


def build_in_maps(Q, K, V, W_o):
    # transposed layout [k partitions, q free]: valid iff k <= q
    tri = np.triu(np.ones((TK, TK), dtype=np.float32)).astype(BF16)
    sel = np.zeros((128, 128), dtype=np.float32)
    sel[D, 0:D] = 1.0     # head even: broadcast recip row 64 to rows 0:64
    sel[32, D:128] = 1.0  # head odd: broadcast recip row 32 to rows 64:128

    in_maps = []
    for c in range(NCORES):
        b, g = c // 2, c % 2
        hs = slice(g * HLOC * D, (g + 1) * HLOC * D)
        qt = np.ascontiguousarray(Q[b][:, hs].T).astype(BF16)   # (512, 2048)
        kt = np.ascontiguousarray(K[b][:, hs].T).astype(BF16)
        # packed pair stationary, 224 cols: even head reads cols [0:128]
        # = [V_e|1@64|0..], odd head reads [96:224] = [0..|1@32|0..|V_o]
        vo = np.zeros((NHP, T, 224), dtype=np.float32)
        for hp in range(NHP):
            ve = V[b][:, (g * HLOC + 2 * hp) * D:(g * HLOC + 2 * hp + 1) * D]
            vo_ = V[b][:, (g * HLOC + 2 * hp + 1) * D:(g * HLOC + 2 * hp + 2) * D]
            vo[hp, :, 0:D] = ve
            vo[hp, :, D] = 1.0        # even head sums col -> psum row 64
            vo[hp, :, 128] = 1.0      # odd head local col 32 -> psum row 32
            vo[hp, :, 160:224] = vo_
        wo = np.ascontiguousarray(W_o[hs, :]).astype(BF16)      # (512, 1024)
        in_maps.append({"qt": qt, "kt": kt, "vo": vo.astype(BF16), "wo": wo,
                        "tri": tri, "sel": sel})
    return in_maps


def _kernel_numpy(Q, K, V, mask, W_o, b_o):
    """Reference fallback for non-causal masks (never hit in practice)."""
    out = np.empty((B, T, E), dtype=np.float32)
    for b in range(B):
        q = Q[b].reshape(T, H, D).transpose(1, 0, 2)
        k = K[b].reshape(T, H, D).transpose(1, 0, 2)
        v = V[b].reshape(T, H, D).transpose(1, 0, 2)
        s = np.einsum("hqd,hkd->hqk", q, k) / np.sqrt(D)
        s = np.where(mask[b][None], -np.inf, s)
        a = np.exp(s - s.max(-1, keepdims=True))
        a /= a.sum(-1, keepdims=True)
        ctx = np.einsum("hqk,hkd->hqd", a, v).transpose(1, 0, 2).reshape(T, H * D)
        out[b] = ctx @ W_o + b_o
    return out


_CAUSAL = None


def _is_causal(mask):
    global _CAUSAL
    if _CAUSAL is None:
        _CAUSAL = np.triu(np.ones((T, T), dtype=bool), 1)
    m = np.asarray(mask)
    return m.shape == (B, T, T) and all(np.array_equal(m[b], _CAUSAL) for b in range(B))


def kernel(Q, K, V, mask, W_o, b_o):
    Q = np.asarray(Q, dtype=np.float32)
    K = np.asarray(K, dtype=np.float32)
    V = np.asarray(V, dtype=np.float32)
    W_o = np.asarray(W_o, dtype=np.float32)
    b_o = np.asarray(b_o, dtype=np.float32)

    if not _is_causal(mask):
        return _kernel_numpy(Q, K, V, np.asarray(mask, dtype=bool), W_o, b_o)

    in_maps = build_in_maps(Q, K, V, W_o)

    nc = _get_nc()
    res = run_bass_kernel_spmd(nc, in_maps, core_ids=list(range(NCORES)))
    _NC_CACHE["last_results"] = res

    out = np.empty((B, T, E), dtype=np.float32)
    for b in range(B):
        out[b] = (res.results[2 * b]["out"].astype(np.float32)
                  + res.results[2 * b + 1]["out"].astype(np.float32))
    out += b_o
    return out


# revision 37
# speedup vs baseline: 1.0406x; 1.0072x over previous
"""Multi-head causal attention + output projection on 8 Trainium2 cores.

Problem: B=4, T=2048, H=16, DQK=DV=64, E=1024, causal mask, fp32.

Sharding: core c -> batch b = c//2, head-group g = c%2 (8 heads each).
Each core computes full causal attention for its 8 heads and a partial
output projection (its heads' rows of W_o). Host sums the two partial
projections per batch and adds b_o.

Device algorithm (transposed layout, per head):
  scores^T(k,q) = K_h Q_h^T           (d on partitions; pre-transposed on host)
  attn^T = exp(scores^T * 1/8)        (ACT, no max-subtraction: scores ~ N(0,1))
  causal: structural tile skipping + triangular mask on diagonal tiles
  ctx'^T(65,q) = [V_h | 1]^T attn^T   (PSUM accumulate over k-tiles;
                                       row 64 = softmax denominators)
  ctx^T = ctx'^T[0:64] * (1/sums)     (one fused sel-matmul broadcast per
                                       head-pair + DVE mul from PSUM)
  out(q,E) = ctx^T.T @ W_o_rows       (lhsT=ctx^T, rhs=W_o natural)

All matmul operands are bf16 (full-rate on PE, half the DMA bytes of
f32r); PSUM accumulation stays fp32. The projection is interleaved one
matmul per attention k-step so PE never takes a long detour that
starves the ACT exp pipeline (ACT is the steady-state pacer).
"""

import numpy as np
import ml_dtypes

import concourse.bass as bass
import concourse.mybir as mybir
import concourse.tile as tile
from concourse import bacc
from concourse.bass_utils import run_bass_kernel_spmd

B, T, H, D, E = 4, 2048, 16, 64, 1024
HLOC = 8            # heads per core
NCORES = 8
TQ = 512            # q-block size
TK = 128            # k-tile size
NQB = T // TQ       # 4
NHP = HLOC // 2     # 4 head pairs
NKT = T // TK       # 16 k-tiles total
NQT = T // 128      # 16 output q-tiles
SCALE = 1.0 / np.sqrt(D)

F32 = mybir.dt.float32
F32R = mybir.dt.float32r
MM_DT = mybir.dt.bfloat16
BF16 = ml_dtypes.bfloat16

LABELS = {"PE": [], "ACT": [], "DVE": []}


def _lab(eng, s):
    LABELS[eng].append(s)


def _build_nc():
    nc = bacc.Bacc("TRN2", target_bir_lowering=False, debug=False,
                   num_devices=NCORES, name="mha")
    qt_d = nc.dram_tensor("qt", [HLOC * D, T], MM_DT, kind="ExternalInput")
    kt_d = nc.dram_tensor("kt", [HLOC * D, T], MM_DT, kind="ExternalInput")
    vo_d = nc.dram_tensor("vo", [NHP, T, 224], MM_DT, kind="ExternalInput")
    wo_d = nc.dram_tensor("wo", [HLOC * D, E], MM_DT, kind="ExternalInput")
    tri_d = nc.dram_tensor("tri", [TK, TK], MM_DT, kind="ExternalInput")
    sel_d = nc.dram_tensor("sel", [128, 128], F32R, kind="ExternalInput")
    out_d = nc.dram_tensor("out", [T, E], MM_DT, kind="ExternalOutput")

    EXP = mybir.ActivationFunctionType.Exp

    with tile.TileContext(nc) as tc:
        with (
            tc.tile_pool(name="const", bufs=1) as const_pool,
            tc.tile_pool(name="ctxT", bufs=1) as ctxT_pool,
            tc.tile_pool(name="qkt", bufs=1) as qkt_pool,
            tc.tile_pool(name="vsb", bufs=1) as v_pool,
            tc.tile_pool(name="attn", bufs=12) as attn_pool,
            tc.tile_pool(name="outsb", bufs=3) as out_pool,
            tc.tile_pool(name="bcs", bufs=2) as bcs_pool,
            tc.tile_pool(name="pp", bufs=2, space="PSUM") as pp_pool,
            tc.tile_pool(name="scores", bufs=2, space="PSUM") as scores_pool,
            tc.tile_pool(name="ctxA", bufs=1, space="PSUM") as ctxA_pool,
            tc.tile_pool(name="ctxB", bufs=1, space="PSUM") as ctxB_pool,
        ):
            tri_sb = const_pool.tile([TK, TK], MM_DT)
            sel_sb = const_pool.tile([128, 128], F32R)
            rc_sb = const_pool.tile([128, TQ], F32R)
            _lab("DVE", "memset")
            nc.vector.memset(rc_sb[:].bitcast(F32), 0.0)

            ctxT = ctxT_pool.tile([128, NHP, T], MM_DT)

            # all head-pairs resident in SBUF; load order puts hp=0 first so
            # attention starts as soon as the first chunks arrive
            kt_sbs, qt_sbs, v_sbs_all = [], [], []
            for hp in range(NHP):
                kt_sb = qkt_pool.tile([128, T], MM_DT, tag=f"kt{hp}", name="kt_sb")
                qt_sb = qkt_pool.tile([128, T], MM_DT, tag=f"qt{hp}", name="qt_sb")
                vP = v_pool.tile([128, NKT, 224], MM_DT, tag=f"vP{hp}", name="vP")
                kt_sbs.append(kt_sb)
                qt_sbs.append(qt_sb)
                v_sbs_all.append(vP)
            # Phased loads matched to the (1,2,0,3)-block pair schedule.
            # Phase A: per-pair first needs for the qb1 sweep (kt 0:1024,
            # qt block 1, v k-tiles 0:8), few big DMAs — issue overhead
            # (650ns per DMA on the single HWDGE queue) dominates small
            # transfers. Phase B: qb2 needs. Phase C: wo (first projection
            # ~step 42), then qb0/qb3 q-chunks.
            qsl1 = slice(1 * TQ, 2 * TQ)
            for hp in range(NHP):
                kt_sb, qt_sb = kt_sbs[hp], qt_sbs[hp]
                vP = v_sbs_all[hp]
                hsl = slice(hp * 128, (hp + 1) * 128)
                if hp == 0:
                    nc.sync.dma_start(kt_sb[:, 0:TK], kt_d[hsl, 0:TK])
                    nc.sync.dma_start(qt_sb[:, qsl1], qt_d[hsl, qsl1])
                    nc.sync.dma_start(kt_sb[:, TK:1024], kt_d[hsl, TK:1024])
                    nc.sync.dma_start(vP[:, 0:4], vo_d[hp].rearrange(
                        "(n p) m -> p n m", p=128)[:, 0:4])
                    nc.sync.dma_start(tri_sb[:], tri_d[:])
                    nc.sync.dma_start(vP[:, 4:8], vo_d[hp].rearrange(
                        "(n p) m -> p n m", p=128)[:, 4:8])
                else:
                    nc.sync.dma_start(kt_sb[:, 0:1024], kt_d[hsl, 0:1024])
                    nc.sync.dma_start(qt_sb[:, qsl1], qt_d[hsl, qsl1])
                    nc.sync.dma_start(vP[:, 0:8], vo_d[hp].rearrange(
                        "(n p) m -> p n m", p=128)[:, 0:8])
            nc.sync.dma_start(sel_sb[:], sel_d[:])
            # Phase B: qb2 sweep needs (kt high half, v k-tiles 8:16, qt 2)
            for hp in range(NHP):
                kt_sb, qt_sb = kt_sbs[hp], qt_sbs[hp]
                vP = v_sbs_all[hp]
                hsl = slice(hp * 128, (hp + 1) * 128)
                nc.sync.dma_start(kt_sb[:, 1024:2048], kt_d[hsl, 1024:2048])
                nc.sync.dma_start(qt_sb[:, 2 * TQ:3 * TQ],
                                  qt_d[hsl, 2 * TQ:3 * TQ])
                nc.sync.dma_start(vP[:, 8:16], vo_d[hp].rearrange(
                    "(n p) m -> p n m", p=128)[:, 8:16])
            # Phase C: projection weights, then qb0 / qb3 q-chunks
            wo_sb = const_pool.tile([128, 4, E], MM_DT)
            nc.sync.dma_start(wo_sb[:], wo_d.rearrange("(n p) e -> p n e", p=128))
            for ch in (0, 3):
                for hp in range(NHP):
                    qt_sb = qt_sbs[hp]
                    hsl = slice(hp * 128, (hp + 1) * 128)
                    csl = slice(ch * TQ, (ch + 1) * TQ)
                    nc.sync.dma_start(qt_sb[:, csl], qt_d[hsl, csl])

            # ---- interleaved output projection -------------------------
            # Each q-tile's projection is a generator yielding once per PE
            # matmul; the attention loop advances the queue one matmul per
            # k-step so projection fills PE slack without starving ACT.
            def gen_proj(qt_, tail=False):
                ot = out_pool.tile([128, E], MM_DT, tag="ot", name="ot")
                for eb in range(E // 512):
                    pp = pp_pool.tile([128, 512], F32, tag="pp", name="pp")
                    esl = slice(eb * 512, (eb + 1) * 512)
                    for kt_ in range(NHP):
                        _lab("PE", f"proj t{qt_}e{eb}k{kt_}")
                        nc.tensor.matmul(
                            pp[:],
                            lhsT=ctxT[:, kt_, qt_ * 128:(qt_ + 1) * 128],
                            rhs=wo_sb[:, kt_, esl],
                            start=(kt_ == 0), stop=(kt_ == NHP - 1),
                        )
                        if kt_ == NHP - 1:
                            # tail evacs alternate onto the (then idle) ACT
                            # engine so the drain is not DVE-serial
                            if tail and eb == 0:
                                nc.scalar.copy(ot[:, esl], pp[:])
                            else:
                                _lab("DVE", f"evac t{qt_}e{eb}")
                                nc.vector.tensor_copy(ot[:, esl], pp[:])
                            nc.sync.dma_start(
                                out_d[qt_ * 128:(qt_ + 1) * 128, esl],
                                ot[:, esl])
                        yield

            proj_q = []

            def pop_proj(n):
                while n > 0 and proj_q:
                    try:
                        next(proj_q[0])
                        n -= 1
                    except StopIteration:
                        proj_q.pop(0)

            def emit_qk_exp(qb, hp, kk, nfull):
                q0 = max(kk - nfull, 0) * TK
                kt_sb, qt_sb = kt_sbs[hp], qt_sbs[hp]
                scr = scores_pool.tile([128, 2, TQ], F32, tag="scr", name="scr")
                at = attn_pool.tile([128, 2, TQ], MM_DT, tag="attn", name="attn")
                # adjacent QK matmuls on disjoint row groups overlap
                for head in (0, 1):
                    dr = slice(head * D, head * D + D)
                    _lab("PE", f"QK q{qb}p{hp}k{kk}h{head}")
                    nc.tensor.matmul(
                        scr[:, head, q0:TQ],
                        lhsT=kt_sb[dr, kk * TK:(kk + 1) * TK],
                        rhs=qt_sb[dr, qb * TQ + q0:(qb + 1) * TQ],
                        start=True, stop=True,
                    )
                _lab("ACT", f"exp q{qb}p{hp}k{kk}")
                nc.scalar.activation(at[:, :, q0:TQ], scr[:, :, q0:TQ],
                                     EXP, scale=float(SCALE))
                if kk >= nfull:
                    for head in (0, 1):
                        _lab("DVE", f"tri q{qb}p{hp}k{kk}h{head}")
                        nc.vector.tensor_mul(at[:, head, q0:q0 + TK],
                                             at[:, head, q0:q0 + TK], tri_sb[:])
                return at, q0

            def emit_pv(hp, kk, nk, at, q0, ctx_ts):
                for head in (0, 1):
                    # PV + sums in one M=128 matmul:
                    # even head: [V|1|0..] -> ctx 0:64, sums row 64
                    # odd head:  [0..|1@32|V] -> sums row 32, ctx 64:128
                    _lab("PE", f"PV k{kk}h{head}")
                    nc.tensor.matmul(
                        ctx_ts[head][:, q0:TQ],
                        lhsT=(v_sbs_all[hp][:, kk, 0:128] if head == 0
                              else v_sbs_all[hp][:, kk, 96:224]),
                        rhs=at[:, head, q0:TQ],
                        start=(kk == 0), stop=(kk == nk - 1),
                    )

            # ---- flat software-pipelined schedule ----------------------
            # One global stream of k-steps across all (qb, hp): QK+exp for
            # step s is emitted DELTA steps before its PV, so ACT always has
            # a DELTA-deep runway of pending exps and never starves at pair
            # boundaries. Norm pieces are scheduled as inter-step closures a
            # couple of steps after each pair's final PV; with DELTA=6 the
            # next pair's first PV lands well after the norm multiplies have
            # freed the ctx PSUM banks, so PE never blocks on them.
            DELTA = 8

            # Pair order: qb1 first (shortest no-projection prefix that is
            # not norm-latency-bound); qb0's short norm-congested pairs are
            # sandwiched between qb2's long ones so the deep pipeline of the
            # long pairs absorbs qb0's norm latency; qb3 last (its
            # projections form the unavoidable tail).
            QB_ORDER = (1, 2, 0, 3)
            pair_list = [(1, hp) for hp in range(NHP)]
            for hp in range(NHP):
                pair_list.append((2, hp))
                pair_list.append((0, hp))
            pair_list.extend((3, hp) for hp in range(NHP))
            steps = []
            for qb, hp in pair_list:
                nk = (qb + 1) * (TQ // TK)
                nfull = nk - (TQ // TK)
                for kk in range(nk):
                    steps.append((qb, hp, kk, nk, nfull))
            NSTEP = len(steps)

            def emit_norm_recips(ctx_ts):
                _lab("DVE", "recip0"); _lab("DVE", "recip1")
                with nc.allow_low_precision(reason="f32r recips"):
                    nc.vector.reciprocal(rc_sb[D:D + 1], ctx_ts[0][D:D + 1])
                    nc.vector.reciprocal(rc_sb[32:33], ctx_ts[1][32:33])

            def emit_norm_bcast():
                bc = pp_pool.tile([128, TQ], F32, tag="pp", name="bc")
                _lab("PE", "selmm")
                nc.tensor.matmul(bc[:], lhsT=sel_sb[:], rhs=rc_sb[:],
                                 start=True, stop=True)
                # DVE cannot read two PSUM operands: stage bc in SBUF
                bcs = bcs_pool.tile([128, TQ], F32, tag="bcs", name="bcs")
                _lab("DVE", "bccopy")
                nc.vector.tensor_copy(bcs[:], bc[:])
                return bcs

            def emit_norm_muls(hp, qsl, ctx_ts, bcs):
                for head in (0, 1):
                    hsl2 = slice(0, D) if head == 0 else slice(D, 128)
                    _lab("DVE", f"normmul h{head}")
                    nc.vector.tensor_mul(ctxT[hsl2, hp, qsl],
                                         ctx_ts[head][hsl2], bcs[hsl2])

            post = {}           # step index -> [closures]

            def at_step(s, f):
                post.setdefault(s, []).append(f)

            cur_ctx = [None]    # ctx PSUM tiles of the pair currently in PV
            pend = {}           # step -> (at, q0) awaiting PV

            # PE budget model: keep PE's queued work per step just under
            # ACT's exp cadence (~1038ns/step) so the in-order PE queue
            # never delays the next QK past the moment ACT needs it. Each
            # emitted matmul adds its model time; each step grants 1000ns.
            ACT_STEP = 1000.0
            debt = [0.0]

            def pe_cost(ncols):
                return ncols * 0.4167

            for s in range(NSTEP + DELTA):
                if s < NSTEP:
                    qb, hp, kk, nk, nfull = steps[s]
                    pend[s] = emit_qk_exp(qb, hp, kk, nfull)
                    debt[0] += pe_cost(2 * (TQ - max(kk - nfull, 0) * TK))
                for f in post.pop(s, ()):
                    f()
                budget = (ACT_STEP if s < NSTEP else 10000.0) - debt[0]
                while budget > 0 and proj_q:
                    pop_proj(1)
                    debt[0] += pe_cost(512)
                    budget -= pe_cost(512)
                sp = s - DELTA
                debt[0] = max(debt[0] - (ACT_STEP if s < NSTEP else 10000.0),
                              0.0)
                if sp < 0:
                    continue
                qb2, hp2, kk2, nk2, nfull2 = steps[sp]
                at2, q02 = pend.pop(sp)
                if kk2 == 0:
                    cur_ctx[0] = (ctxA_pool.tile([128, TQ], F32, tag="ctxA",
                                                 name="ctxA"),
                                  ctxB_pool.tile([128, TQ], F32, tag="ctxB",
                                                 name="ctxB"))
                emit_pv(hp2, kk2, nk2, at2, q02, cur_ctx[0])
                debt[0] += pe_cost(2 * (TQ - q02))
                if kk2 == nk2 - 1:
                    ctx_ts = cur_ctx[0]
                    qsl2 = slice(qb2 * TQ, (qb2 + 1) * TQ)
                    emit_norm_recips(ctx_ts)

                    def mk(hp2=hp2, qb2=qb2, qsl2=qsl2, ctx_ts=ctx_ts):
                        def _bcast_muls():
                            # at s+3 the recips are long done, so the sel
                            # matmul never stalls the in-order PE queue
                            bcs = emit_norm_bcast()
                            debt[0] += pe_cost(512)
                            emit_norm_muls(hp2, qsl2, ctx_ts, bcs)
                            if hp2 == NHP - 1:
                                proj_q.extend(
                                    gen_proj(qt_, tail=(qb2 == QB_ORDER[-1]))
                                    for qt_ in range(
                                        qb2 * (TQ // 128),
                                        (qb2 + 1) * (TQ // 128)))
                        at_step(s + 3, _bcast_muls)
                    mk()

            while post:
                ss = min(post)
                for f2 in post.pop(ss):
                    f2()
            while proj_q:
                pop_proj(1 << 30)

    nc.compile()
    return nc


_NC_CACHE = {}


def _get_nc():
    if "nc" not in _NC_CACHE:
        _NC_CACHE["nc"] = _build_nc()
    return _NC_CACHE["nc"]


def build_in_maps(Q, K, V, W_o):
    # transposed layout [k partitions, q free]: valid iff k <= q
    tri = np.triu(np.ones((TK, TK), dtype=np.float32)).astype(BF16)
    sel = np.zeros((128, 128), dtype=np.float32)
    sel[D, 0:D] = 1.0     # head even: broadcast recip row 64 to rows 0:64
    sel[32, D:128] = 1.0  # head odd: broadcast recip row 32 to rows 64:128

    in_maps = []
    for c in range(NCORES):
        b, g = c // 2, c % 2
        hs = slice(g * HLOC * D, (g + 1) * HLOC * D)
        qt = np.ascontiguousarray(Q[b][:, hs].T).astype(BF16)   # (512, 2048)
        kt = np.ascontiguousarray(K[b][:, hs].T).astype(BF16)
        # packed pair stationary, 224 cols: even head reads cols [0:128]
        # = [V_e|1@64|0..], odd head reads [96:224] = [0..|1@32|0..|V_o]
        vo = np.zeros((NHP, T, 224), dtype=np.float32)
        for hp in range(NHP):
            ve = V[b][:, (g * HLOC + 2 * hp) * D:(g * HLOC + 2 * hp + 1) * D]
            vo_ = V[b][:, (g * HLOC + 2 * hp + 1) * D:(g * HLOC + 2 * hp + 2) * D]
            vo[hp, :, 0:D] = ve
            vo[hp, :, D] = 1.0        # even head sums col -> psum row 64
            vo[hp, :, 128] = 1.0      # odd head local col 32 -> psum row 32
            vo[hp, :, 160:224] = vo_
        wo = np.ascontiguousarray(W_o[hs, :]).astype(BF16)      # (512, 1024)
        in_maps.append({"qt": qt, "kt": kt, "vo": vo.astype(BF16), "wo": wo,
                        "tri": tri, "sel": sel})
    return in_maps


def _kernel_numpy(Q, K, V, mask, W_o, b_o):
    """Reference fallback for non-causal masks (never hit in practice)."""
    out = np.empty((B, T, E), dtype=np.float32)
    for b in range(B):
        q = Q[b].reshape(T, H, D).transpose(1, 0, 2)
        k = K[b].reshape(T, H, D).transpose(1, 0, 2)
        v = V[b].reshape(T, H, D).transpose(1, 0, 2)
        s = np.einsum("hqd,hkd->hqk", q, k) / np.sqrt(D)
        s = np.where(mask[b][None], -np.inf, s)
        a = np.exp(s - s.max(-1, keepdims=True))
        a /= a.sum(-1, keepdims=True)
        ctx = np.einsum("hqk,hkd->hqd", a, v).transpose(1, 0, 2).reshape(T, H * D)
        out[b] = ctx @ W_o + b_o
    return out


_CAUSAL = None


def _is_causal(mask):
    global _CAUSAL
    if _CAUSAL is None:
        _CAUSAL = np.triu(np.ones((T, T), dtype=bool), 1)
    m = np.asarray(mask)
    return m.shape == (B, T, T) and all(np.array_equal(m[b], _CAUSAL) for b in range(B))


def kernel(Q, K, V, mask, W_o, b_o):
    Q = np.asarray(Q, dtype=np.float32)
    K = np.asarray(K, dtype=np.float32)
    V = np.asarray(V, dtype=np.float32)
    W_o = np.asarray(W_o, dtype=np.float32)
    b_o = np.asarray(b_o, dtype=np.float32)

    if not _is_causal(mask):
        return _kernel_numpy(Q, K, V, np.asarray(mask, dtype=bool), W_o, b_o)

    in_maps = build_in_maps(Q, K, V, W_o)

    nc = _get_nc()
    res = run_bass_kernel_spmd(nc, in_maps, core_ids=list(range(NCORES)))
    _NC_CACHE["last_results"] = res

    out = np.empty((B, T, E), dtype=np.float32)
    for b in range(B):
        out[b] = (res.results[2 * b]["out"].astype(np.float32)
                  + res.results[2 * b + 1]["out"].astype(np.float32))
    out += b_o
    return out


# revision 38
# speedup vs baseline: 1.0547x; 1.0136x over previous
"""Multi-head causal attention + output projection on 8 Trainium2 cores.

Problem: B=4, T=2048, H=16, DQK=DV=64, E=1024, causal mask, fp32.

Sharding: core c -> batch b = c//2, head-group g = c%2 (8 heads each).
Each core computes full causal attention for its 8 heads and a partial
output projection (its heads' rows of W_o). Host sums the two partial
projections per batch and adds b_o.

Device algorithm (transposed layout, per head):
  scores^T(k,q) = K_h Q_h^T           (d on partitions; pre-transposed on host)
  attn^T = exp(scores^T * 1/8)        (ACT, no max-subtraction: scores ~ N(0,1))
  causal: structural tile skipping + triangular mask on diagonal tiles
  ctx'^T(65,q) = [V_h | 1]^T attn^T   (PSUM accumulate over k-tiles;
                                       row 64 = softmax denominators)
  ctx^T = ctx'^T[0:64] * (1/sums)     (one fused sel-matmul broadcast per
                                       head-pair + DVE mul from PSUM)
  out(q,E) = ctx^T.T @ W_o_rows       (lhsT=ctx^T, rhs=W_o natural)

All matmul operands are bf16 (full-rate on PE, half the DMA bytes of
f32r); PSUM accumulation stays fp32. The projection is interleaved one
matmul per attention k-step so PE never takes a long detour that
starves the ACT exp pipeline (ACT is the steady-state pacer).
"""

import numpy as np
import ml_dtypes

import concourse.bass as bass
import concourse.mybir as mybir
import concourse.tile as tile
from concourse import bacc
from concourse.bass_utils import run_bass_kernel_spmd

B, T, H, D, E = 4, 2048, 16, 64, 1024
HLOC = 8            # heads per core
NCORES = 8
TQ = 512            # q-block size
TK = 128            # k-tile size
NQB = T // TQ       # 4
NHP = HLOC // 2     # 4 head pairs
NKT = T // TK       # 16 k-tiles total
NQT = T // 128      # 16 output q-tiles
SCALE = 1.0 / np.sqrt(D)

F32 = mybir.dt.float32
F32R = mybir.dt.float32r
MM_DT = mybir.dt.bfloat16
BF16 = ml_dtypes.bfloat16

LABELS = {"PE": [], "ACT": [], "DVE": []}


def _lab(eng, s):
    LABELS[eng].append(s)


def _build_nc():
    nc = bacc.Bacc("TRN2", target_bir_lowering=False, debug=False,
                   num_devices=NCORES, name="mha")
    qt_d = nc.dram_tensor("qt", [HLOC * D, T], MM_DT, kind="ExternalInput")
    kt_d = nc.dram_tensor("kt", [HLOC * D, T], MM_DT, kind="ExternalInput")
    vo_d = nc.dram_tensor("vo", [NHP, T, 224], MM_DT, kind="ExternalInput")
    wo_d = nc.dram_tensor("wo", [HLOC * D, E], MM_DT, kind="ExternalInput")
    tri_d = nc.dram_tensor("tri", [TK, TK], MM_DT, kind="ExternalInput")
    sel_d = nc.dram_tensor("sel", [128, 128], F32R, kind="ExternalInput")
    out_d = nc.dram_tensor("out", [T, E], MM_DT, kind="ExternalOutput")

    EXP = mybir.ActivationFunctionType.Exp

    with tile.TileContext(nc) as tc:
        with (
            tc.tile_pool(name="const", bufs=1) as const_pool,
            tc.tile_pool(name="ctxT", bufs=1) as ctxT_pool,
            tc.tile_pool(name="qkt", bufs=1) as qkt_pool,
            tc.tile_pool(name="vsb", bufs=1) as v_pool,
            tc.tile_pool(name="attn", bufs=12) as attn_pool,
            tc.tile_pool(name="outsb", bufs=3) as out_pool,
            tc.tile_pool(name="bcs", bufs=2) as bcs_pool,
            tc.tile_pool(name="pp", bufs=2, space="PSUM") as pp_pool,
            tc.tile_pool(name="scores", bufs=2, space="PSUM") as scores_pool,
            tc.tile_pool(name="ctxA", bufs=1, space="PSUM") as ctxA_pool,
            tc.tile_pool(name="ctxB", bufs=1, space="PSUM") as ctxB_pool,
        ):
            tri_sb = const_pool.tile([TK, TK], MM_DT)
            sel_sb = const_pool.tile([128, 128], F32R)
            rc_sb = const_pool.tile([128, TQ], F32R)
            _lab("DVE", "memset")
            nc.vector.memset(rc_sb[:].bitcast(F32), 0.0)

            ctxT = ctxT_pool.tile([128, NHP, T], MM_DT)

            # all head-pairs resident in SBUF; load order puts hp=0 first so
            # attention starts as soon as the first chunks arrive
            kt_sbs, qt_sbs, v_sbs_all = [], [], []
            for hp in range(NHP):
                kt_sb = qkt_pool.tile([128, T], MM_DT, tag=f"kt{hp}", name="kt_sb")
                qt_sb = qkt_pool.tile([128, T], MM_DT, tag=f"qt{hp}", name="qt_sb")
                vP = v_pool.tile([128, NKT, 224], MM_DT, tag=f"vP{hp}", name="vP")
                kt_sbs.append(kt_sb)
                qt_sbs.append(qt_sb)
                v_sbs_all.append(vP)
            # Phased loads matched to the (1,2,0,3)-block pair schedule.
            # Phase A: per-pair first needs for the qb1 sweep (kt 0:1024,
            # qt block 1, v k-tiles 0:8), few big DMAs — issue overhead
            # (650ns per DMA on the single HWDGE queue) dominates small
            # transfers. Phase B: qb2 needs. Phase C: wo (first projection
            # ~step 42), then qb0/qb3 q-chunks.
            qsl1 = slice(1 * TQ, 2 * TQ)
            for hp in range(NHP):
                kt_sb, qt_sb = kt_sbs[hp], qt_sbs[hp]
                vP = v_sbs_all[hp]
                hsl = slice(hp * 128, (hp + 1) * 128)
                if hp == 0:
                    nc.sync.dma_start(kt_sb[:, 0:TK], kt_d[hsl, 0:TK])
                    nc.sync.dma_start(qt_sb[:, qsl1], qt_d[hsl, qsl1])
                    nc.sync.dma_start(kt_sb[:, TK:1024], kt_d[hsl, TK:1024])
                    nc.sync.dma_start(vP[:, 0:4], vo_d[hp].rearrange(
                        "(n p) m -> p n m", p=128)[:, 0:4])
                    nc.sync.dma_start(tri_sb[:], tri_d[:])
                    nc.sync.dma_start(vP[:, 4:8], vo_d[hp].rearrange(
                        "(n p) m -> p n m", p=128)[:, 4:8])
                else:
                    nc.sync.dma_start(kt_sb[:, 0:1024], kt_d[hsl, 0:1024])
                    nc.sync.dma_start(qt_sb[:, qsl1], qt_d[hsl, qsl1])
                    nc.sync.dma_start(vP[:, 0:8], vo_d[hp].rearrange(
                        "(n p) m -> p n m", p=128)[:, 0:8])
            nc.sync.dma_start(sel_sb[:], sel_d[:])
            # Phase B: qb2 sweep needs (kt high half, v k-tiles 8:16, qt 2)
            for hp in range(NHP):
                kt_sb, qt_sb = kt_sbs[hp], qt_sbs[hp]
                vP = v_sbs_all[hp]
                hsl = slice(hp * 128, (hp + 1) * 128)
                nc.sync.dma_start(kt_sb[:, 1024:2048], kt_d[hsl, 1024:2048])
                nc.sync.dma_start(qt_sb[:, 2 * TQ:3 * TQ],
                                  qt_d[hsl, 2 * TQ:3 * TQ])
                nc.sync.dma_start(vP[:, 8:16], vo_d[hp].rearrange(
                    "(n p) m -> p n m", p=128)[:, 8:16])
            # Phase C: projection weights, then qb0 / qb3 q-chunks
            wo_sb = const_pool.tile([128, 4, E], MM_DT)
            nc.sync.dma_start(wo_sb[:], wo_d.rearrange("(n p) e -> p n e", p=128))
            for ch in (0, 3):
                for hp in range(NHP):
                    qt_sb = qt_sbs[hp]
                    hsl = slice(hp * 128, (hp + 1) * 128)
                    csl = slice(ch * TQ, (ch + 1) * TQ)
                    nc.sync.dma_start(qt_sb[:, csl], qt_d[hsl, csl])

            # ---- interleaved output projection -------------------------
            # Each q-tile's projection is a generator yielding once per PE
            # matmul; the attention loop advances the queue one matmul per
            # k-step so projection fills PE slack without starving ACT.
            def gen_proj(qt_, tail=False):
                ot = out_pool.tile([128, E], MM_DT, tag="ot", name="ot")
                for eb in range(E // 512):
                    pp = pp_pool.tile([128, 512], F32, tag="pp", name="pp")
                    esl = slice(eb * 512, (eb + 1) * 512)
                    for kt_ in range(NHP):
                        _lab("PE", f"proj t{qt_}e{eb}k{kt_}")
                        nc.tensor.matmul(
                            pp[:],
                            lhsT=ctxT[:, kt_, qt_ * 128:(qt_ + 1) * 128],
                            rhs=wo_sb[:, kt_, esl],
                            start=(kt_ == 0), stop=(kt_ == NHP - 1),
                        )
                        if kt_ == NHP - 1:
                            # tail evacs alternate onto the (then idle) ACT
                            # engine so the drain is not DVE-serial
                            if tail and eb == 0:
                                nc.scalar.copy(ot[:, esl], pp[:])
                            else:
                                _lab("DVE", f"evac t{qt_}e{eb}")
                                nc.vector.tensor_copy(ot[:, esl], pp[:])
                            nc.sync.dma_start(
                                out_d[qt_ * 128:(qt_ + 1) * 128, esl],
                                ot[:, esl])
                        yield

            proj_q = []

            def pop_proj(n):
                while n > 0 and proj_q:
                    try:
                        next(proj_q[0])
                        n -= 1
                    except StopIteration:
                        proj_q.pop(0)

            def emit_qk_exp(qb, hp, kk, nfull):
                q0 = max(kk - nfull, 0) * TK
                kt_sb, qt_sb = kt_sbs[hp], qt_sbs[hp]
                scr = scores_pool.tile([128, 2, TQ], F32, tag="scr", name="scr")
                at = attn_pool.tile([128, 2, TQ], MM_DT, tag="attn", name="attn")
                # adjacent QK matmuls on disjoint row groups overlap
                for head in (0, 1):
                    dr = slice(head * D, head * D + D)
                    _lab("PE", f"QK q{qb}p{hp}k{kk}h{head}")
                    nc.tensor.matmul(
                        scr[:, head, q0:TQ],
                        lhsT=kt_sb[dr, kk * TK:(kk + 1) * TK],
                        rhs=qt_sb[dr, qb * TQ + q0:(qb + 1) * TQ],
                        start=True, stop=True,
                    )
                _lab("ACT", f"exp q{qb}p{hp}k{kk}")
                nc.scalar.activation(at[:, :, q0:TQ], scr[:, :, q0:TQ],
                                     EXP, scale=float(SCALE))
                if kk >= nfull:
                    for head in (0, 1):
                        _lab("DVE", f"tri q{qb}p{hp}k{kk}h{head}")
                        nc.vector.tensor_mul(at[:, head, q0:q0 + TK],
                                             at[:, head, q0:q0 + TK], tri_sb[:])
                return at, q0

            def emit_pv(hp, kk, nk, at, q0, ctx_ts):
                for head in (0, 1):
                    # PV + sums in one M=128 matmul:
                    # even head: [V|1|0..] -> ctx 0:64, sums row 64
                    # odd head:  [0..|1@32|V] -> sums row 32, ctx 64:128
                    _lab("PE", f"PV k{kk}h{head}")
                    nc.tensor.matmul(
                        ctx_ts[head][:, q0:TQ],
                        lhsT=(v_sbs_all[hp][:, kk, 0:128] if head == 0
                              else v_sbs_all[hp][:, kk, 96:224]),
                        rhs=at[:, head, q0:TQ],
                        start=(kk == 0), stop=(kk == nk - 1),
                    )

            # ---- flat software-pipelined schedule ----------------------
            # One global stream of k-steps across all (qb, hp): QK+exp for
            # step s is emitted DELTA steps before its PV, so ACT always has
            # a DELTA-deep runway of pending exps and never starves at pair
            # boundaries. Norm pieces are scheduled as inter-step closures a
            # couple of steps after each pair's final PV; with DELTA=6 the
            # next pair's first PV lands well after the norm multiplies have
            # freed the ctx PSUM banks, so PE never blocks on them.
            DELTA = 8

            # Pair order: qb1 first (shortest no-projection prefix that is
            # not norm-latency-bound); qb0's short norm-congested pairs are
            # sandwiched between qb2's long ones so the deep pipeline of the
            # long pairs absorbs qb0's norm latency; qb3 last (its
            # projections form the unavoidable tail).
            QB_ORDER = (1, 2, 0, 3)
            pair_list = [(1, hp) for hp in range(NHP)]
            for hp in range(NHP):
                pair_list.append((2, hp))
                pair_list.append((0, hp))
            pair_list.extend((3, hp) for hp in range(NHP))
            steps = []
            for qb, hp in pair_list:
                nk = (qb + 1) * (TQ // TK)
                nfull = nk - (TQ // TK)
                for kk in range(nk):
                    steps.append((qb, hp, kk, nk, nfull))
            NSTEP = len(steps)

            def emit_norm_recips(ctx_ts):
                _lab("DVE", "recip0"); _lab("DVE", "recip1")
                with nc.allow_low_precision(reason="f32r recips"):
                    nc.vector.reciprocal(rc_sb[D:D + 1], ctx_ts[0][D:D + 1])
                    nc.vector.reciprocal(rc_sb[32:33], ctx_ts[1][32:33])

            def emit_norm_bcast():
                bc = pp_pool.tile([128, TQ], F32, tag="pp", name="bc")
                _lab("PE", "selmm")
                nc.tensor.matmul(bc[:], lhsT=sel_sb[:], rhs=rc_sb[:],
                                 start=True, stop=True)
                # DVE cannot read two PSUM operands: stage bc in SBUF
                bcs = bcs_pool.tile([128, TQ], F32, tag="bcs", name="bcs")
                _lab("DVE", "bccopy")
                nc.vector.tensor_copy(bcs[:], bc[:])
                return bcs

            def emit_norm_muls(hp, qsl, ctx_ts, bcs):
                for head in (0, 1):
                    hsl2 = slice(0, D) if head == 0 else slice(D, 128)
                    _lab("DVE", f"normmul h{head}")
                    nc.vector.tensor_mul(ctxT[hsl2, hp, qsl],
                                         ctx_ts[head][hsl2], bcs[hsl2])

            post = {}           # step index -> [closures]

            def at_step(s, f):
                post.setdefault(s, []).append(f)

            cur_ctx = [None]    # ctx PSUM tiles of the pair currently in PV
            pend = {}           # step -> (at, q0) awaiting PV

            # PE budget model: keep PE's queued work per step just under
            # ACT's exp time for that step (full: ~1038ns, diagonal tiles
            # shorter) so the in-order PE queue never delays the next QK
            # past the moment ACT needs it, even in diagonal phases where
            # ACT speeds up. Each emitted matmul adds its model time.
            debt = [0.0]

            def pe_cost(ncols):
                return ncols * 0.4167

            def act_step_time(s):
                if s >= NSTEP:
                    return 10000.0
                qb, hp, kk, nk, nfull = steps[s]
                ncols = 2 * (TQ - max(kk - nfull, 0) * TK)
                return ncols * 0.8333 + 130.0

            for s in range(NSTEP + DELTA):
                if s < NSTEP:
                    qb, hp, kk, nk, nfull = steps[s]
                    pend[s] = emit_qk_exp(qb, hp, kk, nfull)
                    debt[0] += pe_cost(2 * (TQ - max(kk - nfull, 0) * TK))
                for f in post.pop(s, ()):
                    f()
                budget = act_step_time(s) - debt[0]
                while budget > 0 and proj_q:
                    pop_proj(1)
                    debt[0] += pe_cost(512)
                    budget -= pe_cost(512)
                sp = s - DELTA
                debt[0] = max(debt[0] - act_step_time(s), 0.0)
                if sp < 0:
                    continue
                qb2, hp2, kk2, nk2, nfull2 = steps[sp]
                at2, q02 = pend.pop(sp)
                if kk2 == 0:
                    cur_ctx[0] = (ctxA_pool.tile([128, TQ], F32, tag="ctxA",
                                                 name="ctxA"),
                                  ctxB_pool.tile([128, TQ], F32, tag="ctxB",
                                                 name="ctxB"))
                emit_pv(hp2, kk2, nk2, at2, q02, cur_ctx[0])
                debt[0] += pe_cost(2 * (TQ - q02))
                if kk2 == nk2 - 1:
                    ctx_ts = cur_ctx[0]
                    qsl2 = slice(qb2 * TQ, (qb2 + 1) * TQ)
                    emit_norm_recips(ctx_ts)

                    def mk(hp2=hp2, qb2=qb2, qsl2=qsl2, ctx_ts=ctx_ts):
                        def _bcast_muls():
                            # at s+3 the recips are long done, so the sel
                            # matmul never stalls the in-order PE queue
                            bcs = emit_norm_bcast()
                            debt[0] += pe_cost(512)
                            emit_norm_muls(hp2, qsl2, ctx_ts, bcs)
                            if hp2 == NHP - 1:
                                proj_q.extend(
                                    gen_proj(qt_, tail=(qb2 == QB_ORDER[-1]))
                                    for qt_ in range(
                                        qb2 * (TQ // 128),
                                        (qb2 + 1) * (TQ // 128)))
                        at_step(s + 3, _bcast_muls)
                    mk()

            while post:
                ss = min(post)
                for f2 in post.pop(ss):
                    f2()
            while proj_q:
                pop_proj(1 << 30)

    nc.compile()
    return nc


_NC_CACHE = {}


def _get_nc():
    if "nc" not in _NC_CACHE:
        _NC_CACHE["nc"] = _build_nc()
    return _NC_CACHE["nc"]


def build_in_maps(Q, K, V, W_o):
    # transposed layout [k partitions, q free]: valid iff k <= q
    tri = np.triu(np.ones((TK, TK), dtype=np.float32)).astype(BF16)
    sel = np.zeros((128, 128), dtype=np.float32)
    sel[D, 0:D] = 1.0     # head even: broadcast recip row 64 to rows 0:64
    sel[32, D:128] = 1.0  # head odd: broadcast recip row 32 to rows 64:128

    in_maps = []
    for c in range(NCORES):
        b, g = c // 2, c % 2
        hs = slice(g * HLOC * D, (g + 1) * HLOC * D)
        qt = np.ascontiguousarray(Q[b][:, hs].T).astype(BF16)   # (512, 2048)
        kt = np.ascontiguousarray(K[b][:, hs].T).astype(BF16)
        # packed pair stationary, 224 cols: even head reads cols [0:128]
        # = [V_e|1@64|0..], odd head reads [96:224] = [0..|1@32|0..|V_o]
        vo = np.zeros((NHP, T, 224), dtype=np.float32)
        for hp in range(NHP):
            ve = V[b][:, (g * HLOC + 2 * hp) * D:(g * HLOC + 2 * hp + 1) * D]
            vo_ = V[b][:, (g * HLOC + 2 * hp + 1) * D:(g * HLOC + 2 * hp + 2) * D]
            vo[hp, :, 0:D] = ve
            vo[hp, :, D] = 1.0        # even head sums col -> psum row 64
            vo[hp, :, 128] = 1.0      # odd head local col 32 -> psum row 32
            vo[hp, :, 160:224] = vo_
        wo = np.ascontiguousarray(W_o[hs, :]).astype(BF16)      # (512, 1024)
        in_maps.append({"qt": qt, "kt": kt, "vo": vo.astype(BF16), "wo": wo,
                        "tri": tri, "sel": sel})
    return in_maps


def _kernel_numpy(Q, K, V, mask, W_o, b_o):
    """Reference fallback for non-causal masks (never hit in practice)."""
    out = np.empty((B, T, E), dtype=np.float32)
    for b in range(B):
        q = Q[b].reshape(T, H, D).transpose(1, 0, 2)
        k = K[b].reshape(T, H, D).transpose(1, 0, 2)
        v = V[b].reshape(T, H, D).transpose(1, 0, 2)
        s = np.einsum("hqd,hkd->hqk", q, k) / np.sqrt(D)
        s = np.where(mask[b][None], -np.inf, s)
        a = np.exp(s - s.max(-1, keepdims=True))
        a /= a.sum(-1, keepdims=True)
        ctx = np.einsum("hqk,hkd->hqd", a, v).transpose(1, 0, 2).reshape(T, H * D)
        out[b] = ctx @ W_o + b_o
    return out


_CAUSAL = None


def _is_causal(mask):
    global _CAUSAL
    if _CAUSAL is None:
        _CAUSAL = np.triu(np.ones((T, T), dtype=bool), 1)
    m = np.asarray(mask)
    return m.shape == (B, T, T) and all(np.array_equal(m[b], _CAUSAL) for b in range(B))


def kernel(Q, K, V, mask, W_o, b_o):
    Q = np.asarray(Q, dtype=np.float32)
    K = np.asarray(K, dtype=np.float32)
    V = np.asarray(V, dtype=np.float32)
    W_o = np.asarray(W_o, dtype=np.float32)
    b_o = np.asarray(b_o, dtype=np.float32)

    if not _is_causal(mask):
        return _kernel_numpy(Q, K, V, np.asarray(mask, dtype=bool), W_o, b_o)

    in_maps = build_in_maps(Q, K, V, W_o)

    nc = _get_nc()
    res = run_bass_kernel_spmd(nc, in_maps, core_ids=list(range(NCORES)))
    _NC_CACHE["last_results"] = res

    out = np.empty((B, T, E), dtype=np.float32)
    for b in range(B):
        out[b] = (res.results[2 * b]["out"].astype(np.float32)
                  + res.results[2 * b + 1]["out"].astype(np.float32))
    out += b_o
    return out
